# revision 1
# baseline (speedup 1.0000x reference)
"""BaseHORN Trainium2 kernel — self-contained.

Problem: harmonic-oscillator RNN over T=784 steps, B=1024, N=256 nodes:
    tanh_t = tanh(s*(W_ih*u[:,t] + b_ih + W_hh@y + b_hh)),  s = 1/sqrt(256)
    x' = x + y;  y' = 0.8*y - x + tanh_t;  out = x_T @ ro_w.T + ro_b

Exact reformulation used on device (verified to machine precision):
    g_{t+1} = 1.8*y_t + tanh_t ;  y_{t+1} = g_{t+1} - g_t   (g_0 = y_0 = 0)
    x_T = g_{T-1}  (telescoping; g_t = x_{t+1})
so only T-1 = 783 recurrence iterations are needed and the x-state never
materializes.  The readout (tiny [1024,256]@[256,10]) runs on host as part
of unsharding.

Sharding: data-parallel across 8 NeuronCores on the batch axis (128 rows
per core), weights replicated.  Within a core the 128 batch columns are
split into two groups of 64 that run skewed by one step so the tensor
engine of one group overlaps the tanh/state-update chain of the other.

Device layout per group: state tiles [128 part, 128 free], free = c*64+b
with node = c*128 + p.  PSUM per (4-step quad, group) is one [128,512]
bank; the input+bias terms for 4 steps are batched into one K=2 matmul
per node-half (ones-row trick: lhsT rows = [W_ih chunk; b_ih+b_hh chunk],
rhs rows = [u_t; 1]).  Matmul operand APs must start at partition
0/32/64, so input rows cycle through those three bases.  Junk matmuls
into a scratch PSUM bank keep the PE HAM activity monitor busy so real
matmuls run at 2.4 GHz instead of the cold 1.2 GHz.
"""

import numpy as np
import ml_dtypes

import concourse.bass as bass
import concourse.bacc as bacc
import concourse.mybir as mybir
import concourse.tile as tile
from concourse.tile_rust import add_dep_helper as _add_dep
from concourse.bass_utils import run_bass_kernel_spmd

F32 = mybir.dt.float32
BF16 = mybir.dt.bfloat16
NP_BF16 = ml_dtypes.bfloat16

B_FULL = 1024
T_FULL = 784
N_ITER = T_FULL - 1          # output = g_{T-1}, produced at iteration T-2
N_CORES = 8
SCALE = float(np.float32(1.0 / np.sqrt(256.0)))


def _build(n_iter=N_ITER, sdt=BF16, ham_warm_n=3, ham_warm_w=512):
    wdt = BF16
    ydt = BF16
    tdt = BF16
    nquad = (n_iter + 3) // 4
    nqblk = (nquad + 2) // 3
    nc = bacc.Bacc("TRN2", target_bir_lowering=False, debug=False,
                   enable_asserts=False, num_devices=N_CORES)

    u4_d = nc.dram_tensor("u4", [6, nqblk * 512], wdt, kind="ExternalInput")
    whh_d = nc.dram_tensor("whh", [128, 512], wdt, kind="ExternalInput")
    win_d = nc.dram_tensor("win", [2, 256], wdt, kind="ExternalInput")
    gout_d = nc.dram_tensor("gout", [128, 256], F32, kind="ExternalOutput")

    with tile.TileContext(nc) as tc:
        with (
            tc.tile_pool(name="const", bufs=1) as cpool,
            tc.tile_pool(name="state_g", bufs=3) as gpool,
            tc.tile_pool(name="state_y", bufs=3) as ypool,
            tc.tile_pool(name="tanh", bufs=3) as tpool,
            tc.tile_pool(name="psA", bufs=2, space="PSUM") as psA,
            tc.tile_pool(name="psB", bufs=2, space="PSUM") as psB,
            tc.tile_pool(name="psJ", bufs=1, space="PSUM") as psJ,
        ):
            u4_s = cpool.tile([66, nqblk * 512], wdt)
            whh_s = cpool.tile([128, 512], wdt)
            win_s = cpool.tile([66, 256], wdt)
            for i, base in enumerate((0, 32, 64)):
                nc.sync.dma_start(u4_s[base:base + 2, :], u4_d[2 * i:2 * i + 2, :])
                nc.sync.dma_start(win_s[base:base + 2, :], win_d[:])
            nc.sync.dma_start(whh_s[:], whh_d[:])

            ps_pools = (psA, psB)
            junk_ps = psJ.tile([128, ham_warm_w], F32, name="junkps") \
                if ham_warm_n else None

            def emit_ham_warm():
                for _ in range(ham_warm_n):
                    nc.tensor.matmul(
                        junk_ps[:, :], whh_s[:, 0:128], whh_s[:, 0:ham_warm_w],
                        start=True, stop=True, skip_group_check=True)

            g_cur = [None, None]
            y_cur = [None, None]
            ps_q = [None, None]
            for g in range(2):
                g_cur[g] = gpool.tile([128, 128], sdt, tag=f"g{g}", name=f"gst{g}")
                y_cur[g] = ypool.tile([128, 128], ydt, tag=f"y{g}", name=f"yst{g}")
                nc.vector.memset(g_cur[g][:], 0.0)
                nc.vector.memset(y_cur[g][:], 0.0)

            def emit_input_mm(g, q):
                ps = ps_pools[g].tile([128, 512], F32, tag=f"ps{g}", name=f"psq{g}")
                base, qblk = 32 * (q % 3), q // 3
                srem = min(4, n_iter - 4 * q)
                ps_r = ps[:, 0:srem * 128].rearrange("p (s i) -> p s i", s=srem)
                u_r = u4_s[base:base + 2,
                           qblk * 512:qblk * 512 + srem * 128]\
                    .rearrange("p (s i) -> p s i", s=srem)
                for ni in range(2):
                    # start=True clears has_written for the WHOLE bank; only
                    # the first matmul touching this psum tile carries it.
                    nc.tensor.matmul(
                        ps_r[:, :, ni * 64:ni * 64 + 64],
                        win_s[base:base + 2, ni * 128:(ni + 1) * 128],
                        u_r[:, :, g * 64:g * 64 + 64],
                        start=(ni == 0), stop=False, skip_group_check=True)
                if ps_q[g] is None or q == 0:
                    ps_q[g] = ps
                return ps

            def emit_w_mms(g, t):
                s = t % 4
                for kc in range(2):
                    for ni in range(2):
                        nc.tensor.matmul(
                            ps_q[g][:, s * 128 + ni * 64:s * 128 + ni * 64 + 64],
                            whh_s[:, (2 * kc + ni) * 128:(2 * kc + ni + 1) * 128],
                            y_cur[g][:, kc * 64:(kc + 1) * 64],
                            start=False, stop=(kc == 1), skip_group_check=True)

            def emit_act_dve(g, t, dep_on=None):
                s = t % 4
                ps = ps_q[g]
                tt = tpool.tile([128, 128], tdt, tag=f"t{g}", name=f"tt{g}")
                nc.scalar.activation(tt[:], ps[:, s * 128:(s + 1) * 128],
                                     mybir.ActivationFunctionType.Tanh,
                                     bias=0.0, scale=SCALE)
                g_new = gpool.tile([128, 128], sdt, tag=f"g{g}", name=f"gst{g}")
                stt = nc.vector.scalar_tensor_tensor(
                    g_new[:], y_cur[g][:], 1.8, tt[:],
                    mybir.AluOpType.mult, mybir.AluOpType.add)
                if dep_on is not None:
                    _add_dep(stt.ins, dep_on.ins, sync=False)
                y_new = ypool.tile([128, 128], ydt, tag=f"y{g}", name=f"yst{g}")
                sub = nc.vector.tensor_sub(y_new[:], g_new[:], g_cur[g][:])
                g_cur[g], y_cur[g] = g_new, y_new
                return sub

            ps_next = [None, None]
            for k in range(n_iter + 1):
                do_a = k < n_iter
                do_b = 1 <= k
                tb = k - 1
                if do_a:
                    if k == 0:
                        emit_input_mm(0, 0)
                    elif k % 4 == 0:
                        ps_q[0] = ps_next[0]
                if do_b:
                    if tb == 0:
                        emit_input_mm(1, 0)
                    elif tb % 4 == 0:
                        ps_q[1] = ps_next[1]
                if do_a:
                    emit_w_mms(0, k)
                if do_b:
                    emit_w_mms(1, tb)
                if do_a and (k + 1) % 4 == 0 and (k + 1) // 4 < nquad \
                        and k + 1 < n_iter:
                    ps_next[0] = emit_input_mm(0, (k + 1) // 4)
                if do_b and (tb + 1) % 4 == 0 and (tb + 1) // 4 < nquad \
                        and tb + 1 < n_iter:
                    ps_next[1] = emit_input_mm(1, (tb + 1) // 4)
                if ham_warm_n:
                    emit_ham_warm()
                tt_a = emit_act_dve(0, k) if do_a else None
                if do_b:
                    emit_act_dve(1, tb, dep_on=tt_a)

            gfin = cpool.tile([128, 256], F32)
            for g in range(2):
                src = g_cur[g]
                if sdt != F32:
                    s32 = cpool.tile([128, 128], F32, name=f"s32_{g}")
                    nc.vector.tensor_copy(s32[:], src[:])
                    src = s32
                for c in range(2):
                    nc.vector.tensor_copy(
                        gfin[:, c * 128 + g * 64:c * 128 + g * 64 + 64],
                        src[:, c * 64:(c + 1) * 64])
            nc.sync.dma_start(gout_d[:], gfin[:])

    nc.compile()
    return nc


def _pack_core(u_slice, W_ih_w, W_ih_b, W_hh_w, W_hh_b, n_iter=N_ITER):
    """u_slice: [128, T] fp32 for one core -> dram input dict (bf16)."""
    nquad = (n_iter + 3) // 4
    nqblk = (nquad + 2) // 3
    u4 = np.zeros((6, nqblk * 512), dtype=np.float32)
    for q in range(nquad):
        base, qblk = 2 * (q % 3), q // 3
        for s in range(min(4, n_iter - 4 * q)):
            col0 = qblk * 512 + s * 128
            u4[base, col0:col0 + 128] = u_slice[:, 4 * q + s]
    u4[1::2, :] = 1.0

    whh = np.zeros((128, 512), dtype=np.float32)
    for kc in range(2):
        for ni in range(2):
            blk = W_hh_w[ni * 128:(ni + 1) * 128, kc * 128:(kc + 1) * 128]
            whh[:, (2 * kc + ni) * 128:(2 * kc + ni + 1) * 128] = blk.T

    win = np.zeros((2, 256), dtype=np.float32)
    win[0, :] = W_ih_w[:, 0]
    win[1, :] = W_ih_b + W_hh_b

    return {
        "u4": u4.astype(NP_BF16),
        "whh": whh.astype(NP_BF16),
        "win": win.astype(NP_BF16),
    }


_NC_CACHE = {}


def kernel(input_sequence, W_ih_w, W_ih_b, W_hh_w, W_hh_b, ro_w, ro_b):
    input_sequence = np.ascontiguousarray(np.asarray(input_sequence, np.float32))
    W_ih_w = np.asarray(W_ih_w, np.float32)
    W_ih_b = np.asarray(W_ih_b, np.float32)
    W_hh_w = np.asarray(W_hh_w, np.float32)
    W_hh_b = np.asarray(W_hh_b, np.float32)
    ro_w = np.asarray(ro_w, np.float32)
    ro_b = np.asarray(ro_b, np.float32)
    B, T = input_sequence.shape
    assert (B, T) == (B_FULL, T_FULL), (B, T)

    if "nc" not in _NC_CACHE:
        _NC_CACHE["nc"] = _build()
    nc = _NC_CACHE["nc"]

    in_maps = []
    for c in range(N_CORES):
        u_slice = input_sequence[128 * c:128 * (c + 1), :]
        in_maps.append(_pack_core(u_slice, W_ih_w, W_ih_b, W_hh_w, W_hh_b))

    res = run_bass_kernel_spmd(nc, in_maps, core_ids=list(range(N_CORES)))

    out = np.empty((B_FULL, 10), dtype=np.float32)
    for c in range(N_CORES):
        gout = res.results[c]["gout"]          # [128p, 256] state layout
        x = np.empty((128, 256), dtype=np.float32)
        x[:, 0:128] = gout[:, 0:128].T         # [b, node 0:128]
        x[:, 128:256] = gout[:, 128:256].T
        out[128 * c:128 * (c + 1)] = x @ ro_w.T + ro_b
    return out


if __name__ == "__main__":
    rng = np.random.default_rng(0)
    u = rng.standard_normal((B_FULL, T_FULL), dtype=np.float32)
    lim1 = np.sqrt(6.0 / 257.0)
    lim2 = np.sqrt(6.0 / 512.0)
    lim3 = np.sqrt(6.0 / 266.0)
    out = kernel(
        u,
        rng.uniform(-lim1, lim1, (256, 1)).astype(np.float32),
        np.zeros(256, np.float32),
        rng.uniform(-lim2, lim2, (256, 256)).astype(np.float32),
        np.zeros(256, np.float32),
        rng.uniform(-lim3, lim3, (10, 256)).astype(np.float32),
        np.zeros(10, np.float32),
    )
    print(out.shape, out.dtype, "nan frac", np.isnan(out).mean())


# revision 2
# speedup vs baseline: 1.0055x; 1.0055x over previous
"""BaseHORN Trainium2 kernel — self-contained.

Problem: harmonic-oscillator RNN over T=784 steps, B=1024, N=256 nodes:
    tanh_t = tanh(s*(W_ih*u[:,t] + b_ih + W_hh@y + b_hh)),  s = 1/sqrt(256)
    x' = x + y;  y' = 0.8*y - x + tanh_t;  out = x_T @ ro_w.T + ro_b

Exact reformulation used on device (verified to machine precision):
    g_{t+1} = 1.8*y_t + tanh_t ;  y_{t+1} = g_{t+1} - g_t   (g_0 = y_0 = 0)
    x_T = g_{T-1}  (telescoping; g_t = x_{t+1})
so only T-1 = 783 recurrence iterations are needed and the x-state never
materializes.  The readout (tiny [1024,256]@[256,10]) runs on host as part
of unsharding.

Sharding: data-parallel across 8 NeuronCores on the batch axis (128 rows
per core), weights replicated.  Within a core the 128 batch columns are
split into two groups of 64 that run skewed by one step so the tensor
engine of one group overlaps the tanh/state-update chain of the other.

Device layout per group: state tiles [128 part, 128 free], free = c*64+b
with node = c*128 + p.  PSUM per (4-step quad, group) is one [128,512]
bank; the input+bias terms for 4 steps are batched into one K=2 matmul
per node-half (ones-row trick: lhsT rows = [W_ih chunk; b_ih+b_hh chunk],
rhs rows = [u_t; 1]).  Matmul operand APs must start at partition
0/32/64, so input rows cycle through those three bases.  Junk matmuls
into a scratch PSUM bank keep the PE HAM activity monitor busy so real
matmuls run at 2.4 GHz instead of the cold 1.2 GHz.
"""

import numpy as np
import ml_dtypes

import concourse.bass as bass
import concourse.bacc as bacc
import concourse.mybir as mybir
import concourse.tile as tile
from concourse.tile_rust import add_dep_helper as _add_dep
from concourse.bass_utils import run_bass_kernel_spmd

F32 = mybir.dt.float32
BF16 = mybir.dt.bfloat16
NP_BF16 = ml_dtypes.bfloat16

B_FULL = 1024
T_FULL = 784
N_ITER = T_FULL - 1          # output = g_{T-1}, produced at iteration T-2
N_CORES = 8
SCALE = float(np.float32(1.0 / np.sqrt(256.0)))


def _build(n_iter=N_ITER, sdt=BF16, ham_warm_n=3, ham_warm_w=512):
    wdt = BF16
    ydt = BF16
    tdt = BF16
    nquad = (n_iter + 3) // 4
    nqblk = (nquad + 2) // 3
    nc = bacc.Bacc("TRN2", target_bir_lowering=False, debug=False,
                   enable_asserts=False, num_devices=N_CORES)

    u4_d = nc.dram_tensor("u4", [6, nqblk * 512], wdt, kind="ExternalInput")
    whh_d = nc.dram_tensor("whh", [128, 512], wdt, kind="ExternalInput")
    win_d = nc.dram_tensor("win", [2, 256], wdt, kind="ExternalInput")
    gout_d = nc.dram_tensor("gout", [128, 256], F32, kind="ExternalOutput")

    with tile.TileContext(nc) as tc:
        with (
            tc.tile_pool(name="const", bufs=1) as cpool,
            tc.tile_pool(name="state_g", bufs=3) as gpool,
            tc.tile_pool(name="state_y", bufs=3) as ypool,
            tc.tile_pool(name="tanh", bufs=3) as tpool,
            tc.tile_pool(name="psA", bufs=2, space="PSUM") as psA,
            tc.tile_pool(name="psB", bufs=2, space="PSUM") as psB,
            tc.tile_pool(name="psJ", bufs=1, space="PSUM") as psJ,
        ):
            u4_s = cpool.tile([66, nqblk * 512], wdt)
            whh_s = cpool.tile([128, 512], wdt)
            win_s = cpool.tile([66, 256], wdt)
            for i, base in enumerate((0, 32, 64)):
                nc.sync.dma_start(u4_s[base:base + 2, :], u4_d[2 * i:2 * i + 2, :])
                nc.sync.dma_start(win_s[base:base + 2, :], win_d[:])
            nc.sync.dma_start(whh_s[:], whh_d[:])

            ps_pools = (psA, psB)
            junk_ps = psJ.tile([128, ham_warm_w], F32, name="junkps") \
                if ham_warm_n else None

            def emit_ham_warm():
                for _ in range(ham_warm_n):
                    nc.tensor.matmul(
                        junk_ps[:, :], whh_s[:, 0:128], whh_s[:, 0:ham_warm_w],
                        start=True, stop=True, skip_group_check=True)

            g_cur = [None, None]
            y_cur = [None, None]
            h_cur = [None, None]          # h = 1.8*y, computed off-chain
            ps_q = [None, None]
            for g in range(2):
                g_cur[g] = gpool.tile([128, 128], sdt, tag=f"g{g}", name=f"gst{g}")
                y_cur[g] = ypool.tile([128, 128], ydt, tag=f"y{g}", name=f"yst{g}")
                h_cur[g] = ypool.tile([128, 128], ydt, tag=f"h{g}", name=f"hst{g}")
                nc.vector.memset(g_cur[g][:], 0.0)
                nc.vector.memset(y_cur[g][:], 0.0)
                nc.vector.memset(h_cur[g][:], 0.0)

            def emit_input_mm(g, q):
                ps = ps_pools[g].tile([128, 512], F32, tag=f"ps{g}", name=f"psq{g}")
                base, qblk = 32 * (q % 3), q // 3
                srem = min(4, n_iter - 4 * q)
                ps_r = ps[:, 0:srem * 128].rearrange("p (s i) -> p s i", s=srem)
                u_r = u4_s[base:base + 2,
                           qblk * 512:qblk * 512 + srem * 128]\
                    .rearrange("p (s i) -> p s i", s=srem)
                for ni in range(2):
                    # start=True clears has_written for the WHOLE bank; only
                    # the first matmul touching this psum tile carries it.
                    nc.tensor.matmul(
                        ps_r[:, :, ni * 64:ni * 64 + 64],
                        win_s[base:base + 2, ni * 128:(ni + 1) * 128],
                        u_r[:, :, g * 64:g * 64 + 64],
                        start=(ni == 0), stop=False, skip_group_check=True)
                if ps_q[g] is None or q == 0:
                    ps_q[g] = ps
                return ps

            def emit_w_mms(g, t):
                s = t % 4
                for kc in range(2):
                    for ni in range(2):
                        nc.tensor.matmul(
                            ps_q[g][:, s * 128 + ni * 64:s * 128 + ni * 64 + 64],
                            whh_s[:, (2 * kc + ni) * 128:(2 * kc + ni + 1) * 128],
                            y_cur[g][:, kc * 64:(kc + 1) * 64],
                            start=False, stop=(kc == 1), skip_group_check=True)

            def emit_act_dve(g, t, dep_on=None):
                s = t % 4
                ps = ps_q[g]
                tt = tpool.tile([128, 128], tdt, tag=f"t{g}", name=f"tt{g}")
                nc.scalar.activation(tt[:], ps[:, s * 128:(s + 1) * 128],
                                     mybir.ActivationFunctionType.Tanh,
                                     bias=0.0, scale=SCALE)
                g_new = gpool.tile([128, 128], sdt, tag=f"g{g}", name=f"gst{g}")
                # g' = h + tanh with h = 1.8*y precomputed off the critical
                # chain (plain TT add runs at DVE 2x; the fused
                # scalar_tensor_tensor with an fp32 immediate is stuck at 1x)
                stt = nc.vector.tensor_tensor(
                    g_new[:], h_cur[g][:], tt[:], mybir.AluOpType.add)
                if dep_on is not None:
                    _add_dep(stt.ins, dep_on.ins, sync=False)
                y_new = ypool.tile([128, 128], ydt, tag=f"y{g}", name=f"yst{g}")
                sub = nc.vector.tensor_sub(y_new[:], g_new[:], g_cur[g][:])
                g_cur[g], y_cur[g] = g_new, y_new
                return sub

            def emit_h_shadow(g):
                h_new = ypool.tile([128, 128], ydt, tag=f"h{g}", name=f"hst{g}")
                nc.vector.tensor_scalar_mul(h_new[:], y_cur[g][:], 1.8)
                h_cur[g] = h_new

            ps_next = [None, None]
            for k in range(n_iter + 1):
                do_a = k < n_iter
                do_b = 1 <= k
                tb = k - 1
                if do_a:
                    if k == 0:
                        emit_input_mm(0, 0)
                    elif k % 4 == 0:
                        ps_q[0] = ps_next[0]
                if do_b:
                    if tb == 0:
                        emit_input_mm(1, 0)
                    elif tb % 4 == 0:
                        ps_q[1] = ps_next[1]
                if do_a:
                    emit_w_mms(0, k)
                if do_b:
                    emit_w_mms(1, tb)
                if do_a and (k + 1) % 4 == 0 and (k + 1) // 4 < nquad \
                        and k + 1 < n_iter:
                    ps_next[0] = emit_input_mm(0, (k + 1) // 4)
                if do_b and (tb + 1) % 4 == 0 and (tb + 1) // 4 < nquad \
                        and tb + 1 < n_iter:
                    ps_next[1] = emit_input_mm(1, (tb + 1) // 4)
                if ham_warm_n:
                    emit_ham_warm()
                tt_a = emit_act_dve(0, k) if do_a else None
                if do_b:
                    emit_act_dve(1, tb, dep_on=tt_a)
                if do_a:
                    emit_h_shadow(0)
                if do_b:
                    emit_h_shadow(1)

            gfin = cpool.tile([128, 256], F32)
            for g in range(2):
                src = g_cur[g]
                if sdt != F32:
                    s32 = cpool.tile([128, 128], F32, name=f"s32_{g}")
                    nc.vector.tensor_copy(s32[:], src[:])
                    src = s32
                for c in range(2):
                    nc.vector.tensor_copy(
                        gfin[:, c * 128 + g * 64:c * 128 + g * 64 + 64],
                        src[:, c * 64:(c + 1) * 64])
            nc.sync.dma_start(gout_d[:], gfin[:])

    nc.compile()
    return nc


def _pack_core(u_slice, W_ih_w, W_ih_b, W_hh_w, W_hh_b, n_iter=N_ITER):
    """u_slice: [128, T] fp32 for one core -> dram input dict (bf16)."""
    nquad = (n_iter + 3) // 4
    nqblk = (nquad + 2) // 3
    u4 = np.zeros((6, nqblk * 512), dtype=np.float32)
    for q in range(nquad):
        base, qblk = 2 * (q % 3), q // 3
        for s in range(min(4, n_iter - 4 * q)):
            col0 = qblk * 512 + s * 128
            u4[base, col0:col0 + 128] = u_slice[:, 4 * q + s]
    u4[1::2, :] = 1.0

    whh = np.zeros((128, 512), dtype=np.float32)
    for kc in range(2):
        for ni in range(2):
            blk = W_hh_w[ni * 128:(ni + 1) * 128, kc * 128:(kc + 1) * 128]
            whh[:, (2 * kc + ni) * 128:(2 * kc + ni + 1) * 128] = blk.T

    win = np.zeros((2, 256), dtype=np.float32)
    win[0, :] = W_ih_w[:, 0]
    win[1, :] = W_ih_b + W_hh_b

    return {
        "u4": u4.astype(NP_BF16),
        "whh": whh.astype(NP_BF16),
        "win": win.astype(NP_BF16),
    }


_NC_CACHE = {}


def kernel(input_sequence, W_ih_w, W_ih_b, W_hh_w, W_hh_b, ro_w, ro_b):
    input_sequence = np.ascontiguousarray(np.asarray(input_sequence, np.float32))
    W_ih_w = np.asarray(W_ih_w, np.float32)
    W_ih_b = np.asarray(W_ih_b, np.float32)
    W_hh_w = np.asarray(W_hh_w, np.float32)
    W_hh_b = np.asarray(W_hh_b, np.float32)
    ro_w = np.asarray(ro_w, np.float32)
    ro_b = np.asarray(ro_b, np.float32)
    B, T = input_sequence.shape
    assert (B, T) == (B_FULL, T_FULL), (B, T)

    if "nc" not in _NC_CACHE:
        _NC_CACHE["nc"] = _build()
    nc = _NC_CACHE["nc"]

    in_maps = []
    for c in range(N_CORES):
        u_slice = input_sequence[128 * c:128 * (c + 1), :]
        in_maps.append(_pack_core(u_slice, W_ih_w, W_ih_b, W_hh_w, W_hh_b))

    res = run_bass_kernel_spmd(nc, in_maps, core_ids=list(range(N_CORES)))

    out = np.empty((B_FULL, 10), dtype=np.float32)
    for c in range(N_CORES):
        gout = res.results[c]["gout"]          # [128p, 256] state layout
        x = np.empty((128, 256), dtype=np.float32)
        x[:, 0:128] = gout[:, 0:128].T         # [b, node 0:128]
        x[:, 128:256] = gout[:, 128:256].T
        out[128 * c:128 * (c + 1)] = x @ ro_w.T + ro_b
    return out


if __name__ == "__main__":
    rng = np.random.default_rng(0)
    u = rng.standard_normal((B_FULL, T_FULL), dtype=np.float32)
    lim1 = np.sqrt(6.0 / 257.0)
    lim2 = np.sqrt(6.0 / 512.0)
    lim3 = np.sqrt(6.0 / 266.0)
    out = kernel(
        u,
        rng.uniform(-lim1, lim1, (256, 1)).astype(np.float32),
        np.zeros(256, np.float32),
        rng.uniform(-lim2, lim2, (256, 256)).astype(np.float32),
        np.zeros(256, np.float32),
        rng.uniform(-lim3, lim3, (10, 256)).astype(np.float32),
        np.zeros(10, np.float32),
    )
    print(out.shape, out.dtype, "nan frac", np.isnan(out).mean())
